# revision 2
# baseline (speedup 1.0000x reference)
"""Trainium2 Bass kernel for batched ADMM sparse-coding iterations (nn_DAD_84507776516366).

V3: float32r matmuls with phi single (12-bit static) and the STATE as an
on-chip fp32r hi/lo pair moving through the PE as one N=2B matmul
(state error feedback to ~24 bits). x0 = y@a runs in native fp32.
2 PE passes/phase; phi DMA identical to the single-pass version.

Math (from the reference):
    d        = diag(inv(L)) * diag(inv(U))  with  P L U = a^T a + rho phi^T phi
    x0       = y @ a
    zmu_0 = 0; u_0 = 0
    for t in 1..10:
        x_hat_t = d * (x0 + zmu_{t-1} @ phi)          (rho = 1)
        mm2_t   = x_hat_t @ phi^T ; fxu_t = mm2_t + u_{t-1}
        c_t     = clamp(fxu_t, -lam, lam)
        u_t     = u_{t-1} + c_t
        zmu_t   = mm2_t - 2 c_t                       (== z_t - u_t)
    return clip(x_hat_10, min(x), max(x))

Sharding: data-parallel over batch, 256 rows per core on 8 cores. a/phi/d
replicated; the small LU runs host-side once. phi is streamed from HBM in
f32 twice per iteration (natural layout for mm1, transposed for mm2).
"""
import numpy as np

RHO = 1.0
LAMDA = 0.1
ADMM_ITERS = 10
N_CORES = 8

B_FULL, M_DIM, A_DIM, S_DIM = 2048, 512, 2048, 6144
B_LOC = B_FULL // N_CORES


def build_program(B, AJ, SM, MK, iters, n_cores=N_CORES, shrink=LAMDA / RHO):
    """Dimensions in units of 128-partitions: A = AJ*128, S = SM*128,
    M = MK*128; B = per-core batch (free dim)."""
    from contextlib import ExitStack
    import concourse.bacc as bacc
    import concourse.tile as tile
    import concourse.mybir as mybir

    F32 = mybir.dt.float32
    F32R = mybir.dt.float32r
    OP = mybir.AluOpType

    KH = 16  # k-tiles per streamed phi tile (1 MB f32)
    assert SM % KH == 0 and AJ == KH
    H = SM // KH

    nc = bacc.Bacc("TRN2", target_bir_lowering=False, debug=False,
                   enable_asserts=False, num_devices=n_cores)

    yT = nc.dram_tensor("yT", [128, MK, B], F32, kind="ExternalInput").ap()
    a_sb = nc.dram_tensor("a_sb", [AJ, 128, MK, 128], F32, kind="ExternalInput").ap()
    phi_f = nc.dram_tensor("phi_f", [AJ, 128, SM, 128], F32R, kind="ExternalInput").ap()
    phiT_f = nc.dram_tensor("phiT_f", [SM, 128, AJ, 128], F32R, kind="ExternalInput").ap()
    d_sb = nc.dram_tensor("d_sb", [128, AJ], F32, kind="ExternalInput").ap()
    mm_sb = nc.dram_tensor("mm_sb", [128, 2], F32, kind="ExternalInput").ap()
    xout = nc.dram_tensor("xout", [128, AJ, B], F32, kind="ExternalOutput").ap()
    u_d = nc.dram_tensor("u_d", [2, SM, 128, B], F32).ap()  # internal scratch

    with tile.TileContext(nc) as tc:
        with ExitStack() as ctx:
            const = ctx.enter_context(tc.tile_pool(name="const", bufs=1))
            state = ctx.enter_context(tc.tile_pool(name="state", bufs=1))
            stream = ctx.enter_context(tc.tile_pool(name="stream", bufs=3))
            ustr = ctx.enter_context(tc.tile_pool(name="ustr", bufs=6))
            tmp = ctx.enter_context(tc.tile_pool(name="tmp", bufs=6))
            ps1 = ctx.enter_context(tc.tile_pool(name="ps1", bufs=4, space="PSUM"))
            ps2 = ctx.enter_context(tc.tile_pool(name="ps2", bufs=3, space="PSUM"))

            d_t = const.tile([128, AJ], F32)
            nc.sync.dma_start(out=d_t[:], in_=d_sb[:])
            mm_t = const.tile([128, 2], F32)
            nc.sync.dma_start(out=mm_t[:], in_=mm_sb[:])
            yT_t = const.tile([128, MK, B], F32)
            nc.sync.dma_start(out=yT_t[:], in_=yT[:])

            x0d = state.tile([128, AJ, B], F32)
            zmu = state.tile([128, SM, 2, B], F32R)
            xh = state.tile([128, AJ, 2, B], F32R)

            def write_pair(hi, lo, src32):
                # hi = fp32r(v); lo = fp32r(v - hi)  (error feedback to ~24b)
                nc.vector.tensor_copy(hi, src32)
                nc.vector.tensor_sub(lo, src32, hi)

            def psum_pair_sum(dst, p):
                # DVE reads only ONE PSUM operand: stage lo half via SBUF.
                s = tmp.tile([128, B], F32, name="fxu", tag="fxu")
                nc.vector.tensor_copy(s[:], p[:, 1, :])
                nc.vector.tensor_add(dst, p[:, 0, :], s[:])

            # ---- prologue: x0d = d * (y @ a)^T ; xh = x0d ----
            for j in range(AJ):
                at = stream.tile([128, KH, 128], F32, name="stream", tag="stream")
                nc.sync.dma_start(out=at[:, :MK, :], in_=a_sb[j])
                p = ps1.tile([128, B], F32, name="p1", tag="p1")
                for k in range(MK):
                    nc.tensor.matmul(p[:], at[:, k, :], yT_t[:, k, :],
                                     start=(k == 0), stop=(k == MK - 1))
                nc.vector.tensor_scalar(x0d[:, j, :], p[:], d_t[:, j:j + 1],
                                        None, OP.mult)
                write_pair(xh[:, j, 0, :], xh[:, j, 1, :], x0d[:, j, :])

            def mm2_phase(first, rb, wb):
                # mm2^T[S,B] = phi @ xh ; then u/zmu updates
                ui_t = {}

                def fetch_u(m):
                    if first or m >= SM:
                        return
                    ui_t[m] = ustr.tile([128, B], F32, name="ustr", tag="ustr")
                    nc.sync.dma_start(out=ui_t[m][:], in_=u_d[rb, m])

                fetch_u(0)
                for m in range(SM):
                    fetch_u(m + 1)
                    pt = stream.tile([128, KH, 128], F32R, name="stream", tag="stream")
                    nc.sync.dma_start(out=pt[:], in_=phiT_f[m])
                    p = ps2.tile([128, 2, B], F32, name="p2", tag="p2")
                    for k in range(AJ):
                        nc.tensor.matmul(p[:, :, :], pt[:, k, :], xh[:, k, :, :],
                                         start=(k == 0), stop=(k == AJ - 1))
                    s32 = tmp.tile([128, B], F32, name="fxu", tag="fxu")
                    psum_pair_sum(s32[:], p)
                    if first:
                        # c = clamp(mm2); u = c; zmu = mm2 - 2c
                        uo = ustr.tile([128, B], F32, name="ustr", tag="ustr")
                        nc.vector.tensor_scalar(uo[:], s32[:], shrink,
                                                -shrink, OP.min, OP.max)
                        nc.sync.dma_start(out=u_d[wb, m], in_=uo[:])
                        zv = tmp.tile([128, B], F32, name="fxu", tag="fxu")
                        nc.vector.scalar_tensor_tensor(zv[:], uo[:],
                                                       -2.0, s32[:], OP.mult, OP.add)
                    else:
                        # fxu = mm2 + u; c = clamp(fxu); zmu = fxu - u - 2c; u += c
                        f = tmp.tile([128, B], F32, name="fxu", tag="fxu")
                        nc.vector.tensor_add(f[:], s32[:], ui_t[m][:])
                        c = tmp.tile([128, B], F32, name="fxu", tag="fxu")
                        nc.vector.tensor_scalar(c[:], f[:], shrink, -shrink,
                                                OP.min, OP.max)
                        t2 = tmp.tile([128, B], F32, name="fxu", tag="fxu")
                        nc.vector.scalar_tensor_tensor(t2[:], c[:], -2.0, f[:],
                                                       OP.mult, OP.add)
                        zv = tmp.tile([128, B], F32, name="fxu", tag="fxu")
                        nc.vector.tensor_sub(zv[:], t2[:], ui_t[m][:])
                        uo = ustr.tile([128, B], F32, name="ustr", tag="ustr")
                        nc.vector.tensor_add(uo[:], ui_t[m][:], c[:])
                        nc.sync.dma_start(out=u_d[wb, m], in_=uo[:])
                    write_pair(zmu[:, m, 0, :], zmu[:, m, 1, :], zv[:])

            def mm1_phase(last):
                # x_hat^T[A,B] = d * (x0^T + phi^T @ zmu)
                for j in range(AJ):
                    p = ps1.tile([128, 2, B], F32, name="p1", tag="p1")
                    for h in range(H):
                        ph = stream.tile([128, KH, 128], F32R, name="stream",
                                         tag="stream")
                        nc.sync.dma_start(out=ph[:],
                                          in_=phi_f[j, :, h * KH:(h + 1) * KH, :])
                        for kk in range(KH):
                            k = h * KH + kk
                            nc.tensor.matmul(p[:, :, :], ph[:, kk, :],
                                             zmu[:, k, :, :],
                                             start=(k == 0), stop=(k == SM - 1))
                    s32 = tmp.tile([128, B], F32, name="fxu", tag="fxu")
                    psum_pair_sum(s32[:], p)
                    if last:
                        t = tmp.tile([128, B], F32, name="fxu", tag="fxu")
                        nc.vector.scalar_tensor_tensor(t[:], s32[:], d_t[:, j:j + 1],
                                                       x0d[:, j, :], OP.mult, OP.add)
                        xo = tmp.tile([128, B], F32, name="fxu", tag="fxu")
                        nc.vector.tensor_scalar(xo[:], t[:], mm_t[:, 0:1],
                                                mm_t[:, 1:2], OP.max, OP.min)
                        nc.sync.dma_start(out=xout[:, j, :], in_=xo[:])
                    else:
                        xv = tmp.tile([128, B], F32, name="fxu", tag="fxu")
                        nc.vector.scalar_tensor_tensor(xv[:], s32[:],
                                                       d_t[:, j:j + 1], x0d[:, j, :],
                                                       OP.mult, OP.add)
                        write_pair(xh[:, j, 0, :], xh[:, j, 1, :], xv[:])

            # ---- iteration 1: mm2 only (zmu_0 = 0 makes mm1 trivial) ----
            mm2_phase(True, 0, 0)
            for i in range(1, max(1, iters - 1)):
                mm1_phase(last=False)
                mm2_phase(False, (i - 1) % 2, i % 2)
            # ---- iteration `iters`: x_hat only, clipped to [min x, max x] ----
            mm1_phase(last=True)

    nc.compile()
    return nc


def _r12(v):
    b = np.ascontiguousarray(v, np.float32).view(np.uint32)
    rb = (b.astype(np.uint64) + 0x800) & 0xFFFFF000
    return np.ascontiguousarray(rb.astype(np.uint32).view(np.float32))


def host_prepare(y, x, a, phi):
    """Host-side prep: LU-derived diagonals + pre-tiled / sharded arrays."""
    import scipy.linalg as sla

    y = np.asarray(y, dtype=np.float32)
    x = np.asarray(x, dtype=np.float32)
    a = np.asarray(a, dtype=np.float32)
    phi = np.asarray(phi, dtype=np.float32)

    B, M = y.shape
    A = a.shape[1]
    S = phi.shape[0]
    AJ, SM, MK = A // 128, S // 128, M // 128
    Bl = B // N_CORES

    m = a.T @ a + np.float32(RHO) * (phi.T @ phi)
    _, L, U = sla.lu(m)
    dL = np.ascontiguousarray(np.diag(np.linalg.inv(L))).astype(np.float32)
    dU = (np.float32(1.0) / np.diag(U)).astype(np.float32)
    d = dL * dU

    d_sb = np.ascontiguousarray(np.broadcast_to(d.reshape(AJ, 128).T, (128, AJ)))
    mm_sb = np.empty((128, 2), np.float32)
    mm_sb[:, 0] = x.min()
    mm_sb[:, 1] = x.max()

    phi_r = phi.reshape(SM, 128, AJ, 128)
    phi_sb = _r12(phi_r.transpose(2, 1, 0, 3))   # [AJ,128,SM,128]
    phiT_sb = _r12(phi_r.transpose(0, 3, 2, 1))  # [SM,128,AJ,128]
    a_sb = np.ascontiguousarray(
        a.reshape(MK, 128, AJ, 128).transpose(2, 1, 0, 3))       # [AJ,128,MK,128]

    in_maps = []
    for c in range(N_CORES):
        yT = np.ascontiguousarray(
            y[c * Bl:(c + 1) * Bl].T.reshape(MK, 128, Bl).transpose(1, 0, 2))
        in_maps.append({
            "yT": yT, "a_sb": a_sb,
            "phi_f": phi_sb, "phiT_f": phiT_sb,
            "d_sb": d_sb, "mm_sb": mm_sb,
        })
    return in_maps


_CACHE = {}


def kernel(y, x, a, phi):
    from concourse.bass_utils import run_bass_kernel_spmd

    y = np.asarray(y, dtype=np.float32)
    B, M = np.asarray(y).shape
    A = np.asarray(a).shape[1]
    S = np.asarray(phi).shape[0]

    key = (B, M, A, S)
    warm = key in _CACHE
    if not warm:
        _CACHE[key] = build_program(B // N_CORES, A // 128, S // 128, M // 128,
                                    ADMM_ITERS)
    nc = _CACHE[key]

    in_maps = host_prepare(y, x, a, phi)
    if not warm:
        # warmup execution: the very first post-compile run has produced
        # corrupted outputs once; discard it and use a fresh execution.
        run_bass_kernel_spmd(nc, in_maps, core_ids=list(range(N_CORES)))
    res = run_bass_kernel_spmd(nc, in_maps, core_ids=list(range(N_CORES)))

    Bl = B // N_CORES
    out = np.empty((B, A), np.float32)
    for c in range(N_CORES):
        r = res.results[c]["xout"]                # [128, AJ, Bl]
        out[c * Bl:(c + 1) * Bl] = r.transpose(2, 1, 0).reshape(Bl, A)
    return out


# revision 3
# speedup vs baseline: 1.0125x; 1.0125x over previous
"""Trainium2 Bass kernel for batched ADMM sparse-coding iterations (nn_DAD_84507776516366).

V3: float32r matmuls with phi single (12-bit static) and the STATE as an
on-chip fp32r hi/lo pair moving through the PE as one N=2B matmul
(state error feedback to ~24 bits). x0 = y@a runs in native fp32.
2 PE passes/phase; phi DMA identical to the single-pass version.

Math (from the reference):
    d        = diag(inv(L)) * diag(inv(U))  with  P L U = a^T a + rho phi^T phi
    x0       = y @ a
    zmu_0 = 0; u_0 = 0
    for t in 1..10:
        x_hat_t = d * (x0 + zmu_{t-1} @ phi)          (rho = 1)
        mm2_t   = x_hat_t @ phi^T ; fxu_t = mm2_t + u_{t-1}
        c_t     = clamp(fxu_t, -lam, lam)
        u_t     = u_{t-1} + c_t
        zmu_t   = mm2_t - 2 c_t                       (== z_t - u_t)
    return clip(x_hat_10, min(x), max(x))

Sharding: data-parallel over batch, 256 rows per core on 8 cores. a/phi/d
replicated; the small LU runs host-side once. phi is streamed from HBM in
f32 twice per iteration (natural layout for mm1, transposed for mm2).
"""
import numpy as np

RHO = 1.0
LAMDA = 0.1
ADMM_ITERS = 10
N_CORES = 8

B_FULL, M_DIM, A_DIM, S_DIM = 2048, 512, 2048, 6144
B_LOC = B_FULL // N_CORES


def build_program(B, AJ, SM, MK, iters, n_cores=N_CORES, shrink=LAMDA / RHO):
    """Dimensions in units of 128-partitions: A = AJ*128, S = SM*128,
    M = MK*128; B = per-core batch (free dim)."""
    from contextlib import ExitStack
    import concourse.bacc as bacc
    import concourse.tile as tile
    import concourse.mybir as mybir

    F32 = mybir.dt.float32
    F32R = mybir.dt.float32r
    OP = mybir.AluOpType

    KH = 16  # k-tiles per streamed phi tile (1 MB f32)
    assert SM % KH == 0 and AJ == KH
    H = SM // KH

    nc = bacc.Bacc("TRN2", target_bir_lowering=False, debug=False,
                   enable_asserts=False, num_devices=n_cores)

    yT = nc.dram_tensor("yT", [128, MK, B], F32, kind="ExternalInput").ap()
    a_sb = nc.dram_tensor("a_sb", [AJ, 128, MK, 128], F32, kind="ExternalInput").ap()
    phi_f = nc.dram_tensor("phi_f", [AJ, 128, SM, 128], F32R, kind="ExternalInput").ap()
    phiT_f = nc.dram_tensor("phiT_f", [SM, 128, AJ, 128], F32R, kind="ExternalInput").ap()
    d_sb = nc.dram_tensor("d_sb", [128, AJ], F32, kind="ExternalInput").ap()
    mm_sb = nc.dram_tensor("mm_sb", [128, 2], F32, kind="ExternalInput").ap()
    xout = nc.dram_tensor("xout", [128, AJ, B], F32, kind="ExternalOutput").ap()
    u_d = nc.dram_tensor("u_d", [2, SM, 128, B], F32).ap()  # internal scratch

    with tile.TileContext(nc) as tc:
        with ExitStack() as ctx:
            const = ctx.enter_context(tc.tile_pool(name="const", bufs=1))
            state = ctx.enter_context(tc.tile_pool(name="state", bufs=1))
            stream = ctx.enter_context(tc.tile_pool(name="stream", bufs=4))
            ustr = ctx.enter_context(tc.tile_pool(name="ustr", bufs=5))
            tmp = ctx.enter_context(tc.tile_pool(name="tmp", bufs=5))
            ps1 = ctx.enter_context(tc.tile_pool(name="ps1", bufs=4, space="PSUM"))
            ps2 = ctx.enter_context(tc.tile_pool(name="ps2", bufs=3, space="PSUM"))

            d_t = const.tile([128, AJ], F32)
            nc.sync.dma_start(out=d_t[:], in_=d_sb[:])
            mm_t = const.tile([128, 2], F32)
            nc.sync.dma_start(out=mm_t[:], in_=mm_sb[:])
            yT_t = const.tile([128, MK, B], F32)
            nc.sync.dma_start(out=yT_t[:], in_=yT[:])

            x0d = state.tile([128, AJ, B], F32)
            zmu = state.tile([128, SM, 2, B], F32R)
            xh = state.tile([128, AJ, 2, B], F32R)

            def write_pair(hi, lo, src32):
                # hi = fp32r(v); lo = fp32r(v - hi)  (error feedback to ~24b)
                nc.vector.tensor_copy(hi, src32)
                nc.vector.tensor_sub(lo, src32, hi)

            def psum_pair_sum(dst, p):
                # DVE reads only ONE PSUM operand: stage lo half via SBUF.
                s = tmp.tile([128, B], F32, name="fxu", tag="fxu")
                nc.vector.tensor_copy(s[:], p[:, 1, :])
                nc.vector.tensor_add(dst, p[:, 0, :], s[:])

            # ---- prologue: x0d = d * (y @ a)^T ; xh = x0d ----
            for j in range(AJ):
                at = stream.tile([128, KH, 128], F32, name="stream", tag="stream")
                nc.sync.dma_start(out=at[:, :MK, :], in_=a_sb[j])
                p = ps1.tile([128, B], F32, name="p1", tag="p1")
                for k in range(MK):
                    nc.tensor.matmul(p[:], at[:, k, :], yT_t[:, k, :],
                                     start=(k == 0), stop=(k == MK - 1))
                nc.vector.tensor_scalar(x0d[:, j, :], p[:], d_t[:, j:j + 1],
                                        None, OP.mult)
                write_pair(xh[:, j, 0, :], xh[:, j, 1, :], x0d[:, j, :])

            def mm2_phase(first, rb, wb, skip_uw=False):
                # mm2^T[S,B] = phi @ xh ; then u/zmu updates
                ui_t = {}

                def fetch_u(m):
                    if first or m >= SM:
                        return
                    ui_t[m] = ustr.tile([128, B], F32, name="ustr", tag="ustr")
                    nc.sync.dma_start(out=ui_t[m][:], in_=u_d[rb, m])

                fetch_u(0)
                for m in range(SM):
                    fetch_u(m + 1)
                    pt = stream.tile([128, KH, 128], F32R, name="stream", tag="stream")
                    nc.sync.dma_start(out=pt[:], in_=phiT_f[m])
                    p = ps2.tile([128, 2, B], F32, name="p2", tag="p2")
                    for k in range(AJ):
                        nc.tensor.matmul(p[:, :, :], pt[:, k, :], xh[:, k, :, :],
                                         start=(k == 0), stop=(k == AJ - 1))
                    s32 = tmp.tile([128, B], F32, name="fxu", tag="fxu")
                    psum_pair_sum(s32[:], p)
                    if first:
                        # c = clamp(mm2); u = c; zmu = mm2 - 2c
                        uo = ustr.tile([128, B], F32, name="ustr", tag="ustr")
                        nc.vector.tensor_scalar(uo[:], s32[:], shrink,
                                                -shrink, OP.min, OP.max)
                        nc.sync.dma_start(out=u_d[wb, m], in_=uo[:])
                        zv = tmp.tile([128, B], F32, name="fxu", tag="fxu")
                        nc.vector.scalar_tensor_tensor(zv[:], uo[:],
                                                       -2.0, s32[:], OP.mult, OP.add)
                    else:
                        # fxu = mm2 + u; c = clamp(fxu); zmu = fxu - u - 2c; u += c
                        f = tmp.tile([128, B], F32, name="fxu", tag="fxu")
                        nc.vector.tensor_add(f[:], s32[:], ui_t[m][:])
                        c = tmp.tile([128, B], F32, name="fxu", tag="fxu")
                        nc.vector.tensor_scalar(c[:], f[:], shrink, -shrink,
                                                OP.min, OP.max)
                        t2 = tmp.tile([128, B], F32, name="fxu", tag="fxu")
                        nc.vector.scalar_tensor_tensor(t2[:], c[:], -2.0, f[:],
                                                       OP.mult, OP.add)
                        zv = tmp.tile([128, B], F32, name="fxu", tag="fxu")
                        nc.vector.tensor_sub(zv[:], t2[:], ui_t[m][:])
                        if not skip_uw:
                            uo = ustr.tile([128, B], F32, name="ustr", tag="ustr")
                            nc.vector.tensor_add(uo[:], ui_t[m][:], c[:])
                            nc.sync.dma_start(out=u_d[wb, m], in_=uo[:])
                    write_pair(zmu[:, m, 0, :], zmu[:, m, 1, :], zv[:])

            def mm1_phase(last):
                # x_hat^T[A,B] = d * (x0^T + phi^T @ zmu)
                for j in range(AJ):
                    p = ps1.tile([128, 2, B], F32, name="p1", tag="p1")
                    for h in range(H):
                        ph = stream.tile([128, KH, 128], F32R, name="stream",
                                         tag="stream")
                        nc.sync.dma_start(out=ph[:],
                                          in_=phi_f[j, :, h * KH:(h + 1) * KH, :])
                        for kk in range(KH):
                            k = h * KH + kk
                            nc.tensor.matmul(p[:, :, :], ph[:, kk, :],
                                             zmu[:, k, :, :],
                                             start=(k == 0), stop=(k == SM - 1))
                    s32 = tmp.tile([128, B], F32, name="fxu", tag="fxu")
                    psum_pair_sum(s32[:], p)
                    if last:
                        t = tmp.tile([128, B], F32, name="fxu", tag="fxu")
                        nc.vector.scalar_tensor_tensor(t[:], s32[:], d_t[:, j:j + 1],
                                                       x0d[:, j, :], OP.mult, OP.add)
                        xo = tmp.tile([128, B], F32, name="fxu", tag="fxu")
                        nc.vector.tensor_scalar(xo[:], t[:], mm_t[:, 0:1],
                                                mm_t[:, 1:2], OP.max, OP.min)
                        nc.sync.dma_start(out=xout[:, j, :], in_=xo[:])
                    else:
                        xv = tmp.tile([128, B], F32, name="fxu", tag="fxu")
                        nc.vector.scalar_tensor_tensor(xv[:], s32[:],
                                                       d_t[:, j:j + 1], x0d[:, j, :],
                                                       OP.mult, OP.add)
                        write_pair(xh[:, j, 0, :], xh[:, j, 1, :], xv[:])

            # ---- iteration 1: mm2 only (zmu_0 = 0 makes mm1 trivial) ----
            mm2_phase(True, 0, 0)
            for i in range(1, max(1, iters - 1)):
                mm1_phase(last=False)
                mm2_phase(False, (i - 1) % 2, i % 2, skip_uw=(i == iters - 2))
            # ---- iteration `iters`: x_hat only, clipped to [min x, max x] ----
            mm1_phase(last=True)

    nc.compile()
    return nc


def _r12(v):
    b = np.ascontiguousarray(v, np.float32).view(np.uint32)
    rb = (b.astype(np.uint64) + 0x800) & 0xFFFFF000
    return np.ascontiguousarray(rb.astype(np.uint32).view(np.float32))


def host_prepare(y, x, a, phi):
    """Host-side prep: LU-derived diagonals + pre-tiled / sharded arrays."""
    import scipy.linalg as sla

    y = np.asarray(y, dtype=np.float32)
    x = np.asarray(x, dtype=np.float32)
    a = np.asarray(a, dtype=np.float32)
    phi = np.asarray(phi, dtype=np.float32)

    B, M = y.shape
    A = a.shape[1]
    S = phi.shape[0]
    AJ, SM, MK = A // 128, S // 128, M // 128
    Bl = B // N_CORES

    m = a.T @ a + np.float32(RHO) * (phi.T @ phi)
    _, L, U = sla.lu(m)
    dL = np.ascontiguousarray(np.diag(np.linalg.inv(L))).astype(np.float32)
    dU = (np.float32(1.0) / np.diag(U)).astype(np.float32)
    d = dL * dU

    d_sb = np.ascontiguousarray(np.broadcast_to(d.reshape(AJ, 128).T, (128, AJ)))
    mm_sb = np.empty((128, 2), np.float32)
    mm_sb[:, 0] = x.min()
    mm_sb[:, 1] = x.max()

    phi_r = phi.reshape(SM, 128, AJ, 128)
    phi_sb = _r12(phi_r.transpose(2, 1, 0, 3))   # [AJ,128,SM,128]
    phiT_sb = _r12(phi_r.transpose(0, 3, 2, 1))  # [SM,128,AJ,128]
    a_sb = np.ascontiguousarray(
        a.reshape(MK, 128, AJ, 128).transpose(2, 1, 0, 3))       # [AJ,128,MK,128]

    in_maps = []
    for c in range(N_CORES):
        yT = np.ascontiguousarray(
            y[c * Bl:(c + 1) * Bl].T.reshape(MK, 128, Bl).transpose(1, 0, 2))
        in_maps.append({
            "yT": yT, "a_sb": a_sb,
            "phi_f": phi_sb, "phiT_f": phiT_sb,
            "d_sb": d_sb, "mm_sb": mm_sb,
        })
    return in_maps


_CACHE = {}


def kernel(y, x, a, phi):
    from concourse.bass_utils import run_bass_kernel_spmd

    y = np.asarray(y, dtype=np.float32)
    B, M = np.asarray(y).shape
    A = np.asarray(a).shape[1]
    S = np.asarray(phi).shape[0]

    key = (B, M, A, S)
    warm = key in _CACHE
    if not warm:
        _CACHE[key] = build_program(B // N_CORES, A // 128, S // 128, M // 128,
                                    ADMM_ITERS)
    nc = _CACHE[key]

    in_maps = host_prepare(y, x, a, phi)
    if not warm:
        # warmup execution: the very first post-compile run has produced
        # corrupted outputs once; discard it and use a fresh execution.
        run_bass_kernel_spmd(nc, in_maps, core_ids=list(range(N_CORES)))
    res = run_bass_kernel_spmd(nc, in_maps, core_ids=list(range(N_CORES)))

    Bl = B // N_CORES
    out = np.empty((B, A), np.float32)
    for c in range(N_CORES):
        r = res.results[c]["xout"]                # [128, AJ, Bl]
        out[c * Bl:(c + 1) * Bl] = r.transpose(2, 1, 0).reshape(Bl, A)
    return out
